# revision 4
# baseline (speedup 1.0000x reference)
"""Distributed Trainium2 kernel for nn_Convblock_72919954751797.

Reference computation (per full input):
    x: (B=8, S=4096, C=512) f32
    w = tanh(einsum('bsc,dck->bkds', x, weights))        # content-dependent taps
    y = x + sum_k shift(x, k-3) * w[k]                   # dynamic depthwise conv
    y = BN1(y)  (stats over (B,S))
    z = gelu_tanh(BN2(y @ conv_kernel))
    out = y + z

Sharding: pure data-parallel over batch (1 sample per core); the only
cross-core traffic is two 4KB AllReduces for the BatchNorm statistics.

On-chip layout is (channel, seq) with channel on partitions. The host
pre-transposes x to (C, S) bf16 and pre-arranges weights into matmul lhsT
layout so the cores do zero layout work.
"""

import sys
import types

sys.path.insert(0, "/opt/trn_rl_repo")

import numpy as np
import ml_dtypes

import concourse.bass as bass
import concourse.tile as tile
from concourse import bacc, mybir
from concourse.bass_utils import run_bass_kernel_spmd

AF = mybir.ActivationFunctionType
ALU = mybir.AluOpType
BF16 = mybir.dt.bfloat16
F32 = mybir.dt.float32

N_CORES = 8
B, S, C, K = 8, 4096, 512, 7
EPS = 1e-5
CC = C // 128          # channel chunks of 128 partitions
SC = 512               # seq-chunk (matmul moving dim)
PAD = 4                # left pad for shift halo (>=3)
HALF = K // 2


def build(s_len=S, n_cores=N_CORES, gelu_fn=None):
    if gelu_fn is None:
        gelu_fn = AF.Gelu_apprx_tanh
    ns = s_len // SC   # number of seq chunks
    inv_n = 1.0 / (n_cores * s_len)

    nc = bacc.Bacc(None, target_bir_lowering=False, num_devices=n_cores)

    xt_ext = nc.declare_dram_parameter("xt", [C, s_len], BF16, isOutput=False)
    wt_ext = nc.declare_dram_parameter("wt", [CC, 128, K, C], BF16, isOutput=False)
    ck_ext = nc.declare_dram_parameter("ck", [CC, 128, C], BF16, isOutput=False)
    bnp_ext = nc.declare_dram_parameter("bnp", [128, 4 * CC], F32, isOutput=False)
    out_ext = nc.declare_dram_parameter("out", [C, s_len], F32, isOutput=True)

    xw = PAD + s_len + PAD  # padded x width

    with tile.TileContext(nc) as tc:
        import contextlib

        ctx = contextlib.ExitStack()
        with ctx:
            pers = ctx.enter_context(tc.tile_pool(name="pers", bufs=1))
            dram = ctx.enter_context(tc.tile_pool(name="dram", bufs=1, space="DRAM"))

            # ---- persistent SBUF tensors ----
            x_cs = [pers.tile([128, xw], BF16, name=f"x_cs{i}", tag=f"x{i}") for i in range(CC)]
            w_sb = [pers.tile([128, K, C], BF16, name=f"w_sb{i}", tag=f"w{i}") for i in range(CC)]
            ck_sb = [pers.tile([128, C], BF16, name=f"ck_sb{i}", tag=f"ck{i}") for i in range(CC)]
            y_sb = [pers.tile([128, s_len], BF16, name=f"y_sb{i}", tag=f"y{i}") for i in range(CC)]
            z_sb = [pers.tile([128, s_len], BF16, name=f"z_sb{i}", tag=f"z{i}") for i in range(CC)]
            bnp = pers.tile([128, 4 * CC], F32, name="bnp", tag="bnp")
            ysum = pers.tile([128, CC, ns], F32, name="ysum", tag="ysum")
            ysq = pers.tile([128, CC, ns], F32, name="ysq", tag="ysq")
            zsum = pers.tile([128, CC, ns], F32, name="zsum", tag="zsum")
            zsq = pers.tile([128, CC, ns], F32, name="zsq", tag="zsq")
            st1 = pers.tile([128, 2, CC], F32, name="st1", tag="st1")
            st1r = pers.tile([128, 2, CC], F32, name="st1r", tag="st1r")
            st2 = pers.tile([128, 2, CC], F32, name="st2", tag="st2")
            st2r = pers.tile([128, 2, CC], F32, name="st2r", tag="st2r")
            fac1 = pers.tile([128, 6, CC], F32, name="fac1", tag="fac1")
            fac2 = pers.tile([128, 6, CC], F32, name="fac2", tag="fac2")
            zero_bias = pers.tile([128, 1], F32, name="zero_bias", tag="zb")

            bounce1i = dram.tile([128, 2 * CC], F32, name="bounce1i", tag="b1i")
            bounce1o = dram.tile([128, 2 * CC], F32, name="bounce1o", tag="b1o")
            bounce2i = dram.tile([128, 2 * CC], F32, name="bounce2i", tag="b2i")
            bounce2o = dram.tile([128, 2 * CC], F32, name="bounce2o", tag="b2o")

            # ---- loads ----
            nc.sync.dma_start(out=bnp, in_=bnp_ext[:, :])
            for cc in range(CC):
                nc.vector.memset(x_cs[cc][:, 0:PAD], 0)
                nc.vector.memset(x_cs[cc][:, PAD + s_len : xw], 0)
                nc.sync.dma_start(
                    out=x_cs[cc][:, PAD : PAD + s_len],
                    in_=xt_ext[cc * 128 : (cc + 1) * 128, :],
                )
                nc.sync.dma_start(out=w_sb[cc], in_=wt_ext[cc])
                nc.sync.dma_start(out=ck_sb[cc], in_=ck_ext[cc])

            nc.vector.memset(zero_bias, 0.0)

            # ---- PASS A: w_pre matmul + tanh + dynamic conv -> y, stats ----
            pa_ctx = tc.tile_pool(name="pa", bufs=3)
            psA_ctx = tc.tile_pool(name="psA", bufs=4, space="PSUM")
            pa = pa_ctx.__enter__()
            psA = psA_ctx.__enter__()

            kgroups = [(0, 2), (2, 4), (4, 6), (6, 7)]
            for isc in range(ns):
                s0 = isc * SC
                for dc in range(CC):
                    wt_t = pa.tile([128, K, SC], BF16, name="wt_t", tag="wt_t")
                    for k0, k1 in kgroups:
                        wp = psA.tile([128, 2, SC], F32, name="wp", tag="wp")
                        for k in range(k0, k1):
                            for cc in range(CC):
                                nc.tensor.matmul(
                                    out=wp[:, k - k0, :],
                                    lhsT=w_sb[cc][:, k, dc * 128 : (dc + 1) * 128],
                                    rhs=x_cs[cc][:, PAD + s0 : PAD + s0 + SC],
                                    start=(cc == 0),
                                    stop=(cc == CC - 1),
                                )
                        nc.scalar.activation(
                            out=wt_t[:, k0:k1, :],
                            in_=wp[:, 0 : k1 - k0, :],
                            func=AF.Tanh,
                        )
                    # dynamic conv: y = x + sum_k x[.,s+k-3] * wt[k]
                    ta = pa.tile([128, SC], BF16, name="ta", tag="ta")
                    tb = pa.tile([128, SC], BF16, name="tb", tag="tb")
                    xsl = lambda k: x_cs[dc][:, PAD + s0 + k - HALF : PAD + s0 + k - HALF + SC]
                    nc.vector.tensor_mul(out=ta, in0=xsl(0), in1=wt_t[:, 0, :])
                    for k in range(1, K):
                        nc.vector.tensor_mul(out=tb, in0=xsl(k), in1=wt_t[:, k, :])
                        nc.vector.tensor_add(out=ta, in0=ta, in1=tb)
                    ysl = y_sb[dc][:, s0 : s0 + SC]
                    nc.vector.scalar_tensor_tensor(
                        out=ysl,
                        in0=ta,
                        scalar=1.0,
                        in1=x_cs[dc][:, PAD + s0 : PAD + s0 + SC],
                        op0=ALU.mult,
                        op1=ALU.add,
                        accum_out=ysum[:, dc, isc : isc + 1],
                    )
                    nc.vector.scalar_tensor_tensor(
                        out=tb,
                        in0=ysl,
                        scalar=1.0,
                        in1=ysl,
                        op0=ALU.mult,
                        op1=ALU.mult,
                        accum_out=ysq[:, dc, isc : isc + 1],
                    )

            psA_ctx.__exit__(None, None, None)
            pa_ctx.__exit__(None, None, None)

            # ---- BN1 stats all-reduce ----
            for dc in range(CC):
                nc.vector.reduce_sum(out=st1[:, 0, dc : dc + 1], in_=ysum[:, dc, :], axis=mybir.AxisListType.X)
                nc.vector.reduce_sum(out=st1[:, 1, dc : dc + 1], in_=ysq[:, dc, :], axis=mybir.AxisListType.X)
            nc.sync.dma_start(out=bounce1i[:, :], in_=st1[:, :, :])
            nc.gpsimd.collective_compute(
                "AllReduce",
                ALU.add,
                replica_groups=[list(range(n_cores))],
                ins=[bounce1i.opt()],
                outs=[bounce1o.opt()],
            )
            nc.sync.dma_start(out=st1r[:, :, :], in_=bounce1o[:, :])

            # factors: mean = sum/n ; var = sq/n - mean^2 ; rg = scale/sqrt(var+eps)
            # bmr = bias - mean*rg    (fac[:,0,:]=rg, fac[:,1,:]=bmr)
            def bn_factors(stR, fac, sc_col, bi_col):
                mean = fac[:, 2, :]
                var = fac[:, 3, :]
                tmp = fac[:, 4, :]
                std = fac[:, 5, :]
                nc.vector.tensor_scalar_mul(out=mean, in0=stR[:, 0, :], scalar1=inv_n)
                nc.vector.tensor_mul(out=tmp, in0=mean, in1=mean)
                nc.vector.tensor_scalar_mul(out=var, in0=stR[:, 1, :], scalar1=inv_n)
                nc.vector.tensor_sub(out=var, in0=var, in1=tmp)
                nc.vector.tensor_scalar_add(out=var, in0=var, scalar1=EPS)
                nc.scalar.activation(out=std, in_=var, func=AF.Sqrt)
                nc.vector.reciprocal(out=tmp, in_=std)
                nc.vector.tensor_mul(
                    out=fac[:, 0, :], in0=tmp, in1=bnp[:, sc_col * CC : (sc_col + 1) * CC]
                )
                nc.vector.tensor_mul(out=tmp, in0=mean, in1=fac[:, 0, :])
                nc.vector.tensor_sub(
                    out=fac[:, 1, :], in0=bnp[:, bi_col * CC : (bi_col + 1) * CC], in1=tmp
                )

            bn_factors(st1r, fac1, 0, 1)

            # ---- normalize y in place (y -> yn) ----
            for dc in range(CC):
                for isc in range(ns):
                    ysl = y_sb[dc][:, isc * SC : (isc + 1) * SC]
                    nc.vector.tensor_scalar(
                        out=ysl,
                        in0=ysl,
                        scalar1=fac1[:, 0, dc : dc + 1],
                        scalar2=fac1[:, 1, dc : dc + 1],
                        op0=ALU.mult,
                        op1=ALU.add,
                    )

            # ---- PASS B: z = yn @ ck ----
            psB_ctx = tc.tile_pool(name="psB", bufs=4, space="PSUM")
            pb_ctx = tc.tile_pool(name="pb", bufs=3)
            psB = psB_ctx.__enter__()
            pb = pb_ctx.__enter__()
            for isc in range(ns):
                s0 = isc * SC
                for oc in range(CC):
                    zp = psB.tile([128, SC], F32, name="zp", tag="zp")
                    for cc in range(CC):
                        nc.tensor.matmul(
                            out=zp,
                            lhsT=ck_sb[cc][:, oc * 128 : (oc + 1) * 128],
                            rhs=y_sb[cc][:, s0 : s0 + SC],
                            start=(cc == 0),
                            stop=(cc == CC - 1),
                        )
                    zsl = z_sb[oc][:, s0 : s0 + SC]
                    nc.scalar.activation(
                        out=zsl,
                        in_=zp,
                        func=AF.Identity,
                        bias=zero_bias[:, 0:1],
                        accum_out=zsum[:, oc, isc : isc + 1],
                    )
                    tb2 = pb.tile([128, SC], BF16, name="tb2", tag="tb2")
                    nc.vector.scalar_tensor_tensor(
                        out=tb2,
                        in0=zsl,
                        scalar=1.0,
                        in1=zsl,
                        op0=ALU.mult,
                        op1=ALU.mult,
                        accum_out=zsq[:, oc, isc : isc + 1],
                    )

            psB_ctx.__exit__(None, None, None)
            pb_ctx.__exit__(None, None, None)

            # ---- BN2 stats all-reduce ----
            for oc in range(CC):
                nc.vector.reduce_sum(out=st2[:, 0, oc : oc + 1], in_=zsum[:, oc, :], axis=mybir.AxisListType.X)
                nc.vector.reduce_sum(out=st2[:, 1, oc : oc + 1], in_=zsq[:, oc, :], axis=mybir.AxisListType.X)
            nc.sync.dma_start(out=bounce2i[:, :], in_=st2[:, :, :])
            nc.gpsimd.collective_compute(
                "AllReduce",
                ALU.add,
                replica_groups=[list(range(n_cores))],
                ins=[bounce2i.opt()],
                outs=[bounce2o.opt()],
            )
            nc.sync.dma_start(out=st2r[:, :, :], in_=bounce2o[:, :])
            bn_factors(st2r, fac2, 2, 3)

            # ---- FINAL: out = yn + gelu(z*rg2 + bmr2) ----
            pf = ctx.enter_context(tc.tile_pool(name="pf", bufs=3))
            for isc in range(ns):
                s0 = isc * SC
                for oc in range(CC):
                    g = pf.tile([128, SC], BF16, name="g", tag="g")
                    nc.scalar.activation(
                        out=g,
                        in_=z_sb[oc][:, s0 : s0 + SC],
                        func=gelu_fn,
                        scale=fac2[:, 0, oc : oc + 1],
                        bias=fac2[:, 1, oc : oc + 1],
                    )
                    o32 = pf.tile([128, SC], F32, name="o32", tag="o32")
                    nc.vector.tensor_add(out=o32, in0=y_sb[oc][:, s0 : s0 + SC], in1=g)
                    nc.sync.dma_start(
                        out=out_ext[oc * 128 : (oc + 1) * 128, s0 : s0 + SC], in_=o32
                    )

    nc.compile()
    return nc


def _host_prep(x, weights, bn1_scale, bn1_bias, conv_kernel, bn2_scale, bn2_bias, s_len=S, n_cores=N_CORES):
    """Pre-layout everything on the host; returns per-core in_maps."""
    bf = ml_dtypes.bfloat16
    # x: (B, S, C) -> per-core (C, S) bf16
    xts = [np.ascontiguousarray(x[i].T).astype(bf) for i in range(n_cores)]
    # weights: (D, C, K) -> [cc, c(128), k, d] bf16 (matmul lhsT layout)
    wt = np.ascontiguousarray(np.transpose(weights, (1, 2, 0))).astype(bf)  # (C, K, D)
    wt = wt.reshape(CC, 128, K, C)
    # conv_kernel: (C, O) -> [cc, c(128), o]
    ck = np.ascontiguousarray(conv_kernel).astype(bf).reshape(CC, 128, C)
    # bn params -> [128, 4*CC] f32: cols [g1 | b1 | g2 | b2], each [128, CC]
    def pack(p):
        return np.ascontiguousarray(p.reshape(CC, 128).T)
    bnp = np.concatenate(
        [pack(bn1_scale), pack(bn1_bias), pack(bn2_scale), pack(bn2_bias)], axis=1
    ).astype(np.float32)
    in_maps = [
        {"xt": xts[i], "wt": wt, "ck": ck, "bnp": bnp} for i in range(n_cores)
    ]
    return in_maps


_NC_CACHE = {}


def kernel(x, weights, bn1_scale, bn1_bias, conv_kernel, bn2_scale, bn2_bias):
    x = np.asarray(x, dtype=np.float32)
    weights = np.asarray(weights, dtype=np.float32)
    bn1_scale = np.asarray(bn1_scale, dtype=np.float32)
    bn1_bias = np.asarray(bn1_bias, dtype=np.float32)
    conv_kernel = np.asarray(conv_kernel, dtype=np.float32)
    bn2_scale = np.asarray(bn2_scale, dtype=np.float32)
    bn2_bias = np.asarray(bn2_bias, dtype=np.float32)

    if "nc" not in _NC_CACHE:
        _NC_CACHE["nc"] = build()
    nc = _NC_CACHE["nc"]

    in_maps = _host_prep(x, weights, bn1_scale, bn1_bias, conv_kernel, bn2_scale, bn2_bias)
    res = run_bass_kernel_spmd(nc, in_maps, list(range(N_CORES)))
    out = np.stack([res.results[i]["out"].T for i in range(N_CORES)], axis=0)
    return np.ascontiguousarray(out.astype(np.float32))


# revision 5
# speedup vs baseline: 1.0471x; 1.0471x over previous
"""Distributed Trainium2 kernel for nn_Convblock_72919954751797.

Reference computation (per full input):
    x: (B=8, S=4096, C=512) f32
    w = tanh(einsum('bsc,dck->bkds', x, weights))        # content-dependent taps
    y = x + sum_k shift(x, k-3) * w[k]                   # dynamic depthwise conv
    y = BN1(y)  (stats over (B,S))
    z = gelu_tanh(BN2(y @ conv_kernel))
    out = y + z

Sharding: pure data-parallel over batch (1 sample per core); the only
cross-core traffic is two 4KB AllReduces for the BatchNorm statistics.

On-chip layout is (channel, seq) with channel on partitions. The host
pre-transposes x to (C, S) bf16 and pre-arranges weights into matmul lhsT
layout so the cores do zero layout work.
"""

import sys
import types

sys.path.insert(0, "/opt/trn_rl_repo")

import numpy as np
import ml_dtypes

import concourse.bass as bass
import concourse.tile as tile
from concourse import bacc, mybir
from concourse.bass_utils import run_bass_kernel_spmd

AF = mybir.ActivationFunctionType
ALU = mybir.AluOpType
BF16 = mybir.dt.bfloat16
F32 = mybir.dt.float32

N_CORES = 8
B, S, C, K = 8, 4096, 512, 7
EPS = 1e-5
CC = C // 128          # channel chunks of 128 partitions
SC = 512               # seq-chunk (matmul moving dim)
PAD = 4                # left pad for shift halo (>=3)
HALF = K // 2


def build(s_len=S, n_cores=N_CORES, gelu_fn=None):
    if gelu_fn is None:
        gelu_fn = AF.Gelu_apprx_tanh
    ns = s_len // SC   # number of seq chunks
    inv_n = 1.0 / (n_cores * s_len)

    nc = bacc.Bacc(None, target_bir_lowering=False, num_devices=n_cores)

    xt_ext = nc.declare_dram_parameter("xt", [C, s_len], BF16, isOutput=False)
    wt_ext = nc.declare_dram_parameter("wt", [CC, 128, K, C], BF16, isOutput=False)
    ck_ext = nc.declare_dram_parameter("ck", [CC, 128, C], BF16, isOutput=False)
    bnp_ext = nc.declare_dram_parameter("bnp", [128, 4 * CC], F32, isOutput=False)
    out_ext = nc.declare_dram_parameter("out", [C, s_len], F32, isOutput=True)

    xw = PAD + s_len + PAD  # padded x width

    with tile.TileContext(nc) as tc:
        import contextlib

        ctx = contextlib.ExitStack()
        with ctx:
            pers = ctx.enter_context(tc.tile_pool(name="pers", bufs=1))
            dram = ctx.enter_context(tc.tile_pool(name="dram", bufs=1, space="DRAM"))

            # ---- persistent SBUF tensors ----
            x_cs = [pers.tile([128, xw], BF16, name=f"x_cs{i}", tag=f"x{i}") for i in range(CC)]
            w_sb = [pers.tile([128, K, C], BF16, name=f"w_sb{i}", tag=f"w{i}") for i in range(CC)]
            ck_sb = [pers.tile([128, C], BF16, name=f"ck_sb{i}", tag=f"ck{i}") for i in range(CC)]
            y_sb = [pers.tile([128, s_len], BF16, name=f"y_sb{i}", tag=f"y{i}") for i in range(CC)]
            z_sb = [pers.tile([128, s_len], BF16, name=f"z_sb{i}", tag=f"z{i}") for i in range(CC)]
            bnp = pers.tile([128, 4 * CC], F32, name="bnp", tag="bnp")
            ysum = pers.tile([128, CC, ns], F32, name="ysum", tag="ysum")
            ysq = pers.tile([128, CC, ns], F32, name="ysq", tag="ysq")
            zsum = pers.tile([128, CC, ns], F32, name="zsum", tag="zsum")
            zsq = pers.tile([128, CC, ns], F32, name="zsq", tag="zsq")
            st1 = pers.tile([128, 2, CC], F32, name="st1", tag="st1")
            st1r = pers.tile([128, 2, CC], F32, name="st1r", tag="st1r")
            st2 = pers.tile([128, 2, CC], F32, name="st2", tag="st2")
            st2r = pers.tile([128, 2, CC], F32, name="st2r", tag="st2r")
            fac1 = pers.tile([128, 6, CC], F32, name="fac1", tag="fac1")
            fac2 = pers.tile([128, 6, CC], F32, name="fac2", tag="fac2")
            zero_bias = pers.tile([128, 1], F32, name="zero_bias", tag="zb")

            bounce1i = dram.tile([128, 2 * CC], F32, name="bounce1i", tag="b1i")
            bounce1o = dram.tile([128, 2 * CC], F32, name="bounce1o", tag="b1o")
            bounce2i = dram.tile([128, 2 * CC], F32, name="bounce2i", tag="b2i")
            bounce2o = dram.tile([128, 2 * CC], F32, name="bounce2o", tag="b2o")

            # ---- loads ----
            nc.sync.dma_start(out=bnp, in_=bnp_ext[:, :])
            for cc in range(CC):
                nc.vector.memset(x_cs[cc][:, 0:PAD], 0)
                nc.vector.memset(x_cs[cc][:, PAD + s_len : xw], 0)
                nc.sync.dma_start(
                    out=x_cs[cc][:, PAD : PAD + s_len],
                    in_=xt_ext[cc * 128 : (cc + 1) * 128, :],
                )
                nc.sync.dma_start(out=w_sb[cc], in_=wt_ext[cc])
                nc.sync.dma_start(out=ck_sb[cc], in_=ck_ext[cc])

            nc.vector.memset(zero_bias, 0.0)

            # ---- PASS A: w_pre matmul + tanh + dynamic conv -> y, stats ----
            pa_ctx = tc.tile_pool(name="pa", bufs=2)
            psA_ctx = tc.tile_pool(name="psA", bufs=2, space="PSUM")
            pa = pa_ctx.__enter__()
            psA = psA_ctx.__enter__()

            # loop: (dc, s-half, k) rounds; each round = 4 LDW + 16 MMs over
            # 4 seq-chunks, tanh drains 4 PSUM banks in one ACT op.
            SH = 4          # seq-chunks per half-group
            nsh = max(1, ns // SH)
            sh_chunks = [list(range(h * SH, min((h + 1) * SH, ns))) for h in range(nsh)]
            for dc in range(CC):
                for chunks in sh_chunks:
                    wt_t = pa.tile([128, K, SH, SC], BF16, name="wt_t", tag="wt_t")
                    for k in range(K):
                        wp = psA.tile([128, SH, SC], F32, name="wp", tag="wp")
                        for cc in range(CC):
                            for j, isc in enumerate(chunks):
                                s0 = isc * SC
                                nc.tensor.matmul(
                                    out=wp[:, j, :],
                                    lhsT=w_sb[cc][:, k, dc * 128 : (dc + 1) * 128],
                                    rhs=x_cs[cc][:, PAD + s0 : PAD + s0 + SC],
                                    start=(cc == 0),
                                    stop=(cc == CC - 1),
                                )
                        nc.scalar.activation(
                            out=wt_t[:, k, 0 : len(chunks), :],
                            in_=wp[:, 0 : len(chunks), :],
                            func=AF.Tanh,
                        )
                    # dynamic conv: y = x + sum_k x[.,s+k-3] * wt[k]
                    for j, isc in enumerate(chunks):
                        s0 = isc * SC
                        ta = pa.tile([128, SC], BF16, name="ta", tag="ta")
                        tb = pa.tile([128, SC], BF16, name="tb", tag="tb")
                        xsl = lambda k: x_cs[dc][:, PAD + s0 + k - HALF : PAD + s0 + k - HALF + SC]
                        nc.vector.tensor_mul(out=ta, in0=xsl(0), in1=wt_t[:, 0, j, :])
                        for k in range(1, K):
                            nc.vector.tensor_mul(out=tb, in0=xsl(k), in1=wt_t[:, k, j, :])
                            nc.vector.tensor_add(out=ta, in0=ta, in1=tb)
                        ysl = y_sb[dc][:, s0 : s0 + SC]
                        nc.vector.scalar_tensor_tensor(
                            out=ysl,
                            in0=ta,
                            scalar=1.0,
                            in1=x_cs[dc][:, PAD + s0 : PAD + s0 + SC],
                            op0=ALU.mult,
                            op1=ALU.add,
                            accum_out=ysum[:, dc, isc : isc + 1],
                        )
                        nc.vector.scalar_tensor_tensor(
                            out=tb,
                            in0=ysl,
                            scalar=1.0,
                            in1=ysl,
                            op0=ALU.mult,
                            op1=ALU.mult,
                            accum_out=ysq[:, dc, isc : isc + 1],
                        )

            psA_ctx.__exit__(None, None, None)
            pa_ctx.__exit__(None, None, None)

            # ---- BN1 stats all-reduce ----
            for dc in range(CC):
                nc.vector.reduce_sum(out=st1[:, 0, dc : dc + 1], in_=ysum[:, dc, :], axis=mybir.AxisListType.X)
                nc.vector.reduce_sum(out=st1[:, 1, dc : dc + 1], in_=ysq[:, dc, :], axis=mybir.AxisListType.X)
            nc.sync.dma_start(out=bounce1i[:, :], in_=st1[:, :, :])
            nc.gpsimd.collective_compute(
                "AllReduce",
                ALU.add,
                replica_groups=[list(range(n_cores))],
                ins=[bounce1i.opt()],
                outs=[bounce1o.opt()],
            )
            nc.sync.dma_start(out=st1r[:, :, :], in_=bounce1o[:, :])

            # factors: mean = sum/n ; var = sq/n - mean^2 ; rg = scale/sqrt(var+eps)
            # bmr = bias - mean*rg    (fac[:,0,:]=rg, fac[:,1,:]=bmr)
            def bn_factors(stR, fac, sc_col, bi_col):
                mean = fac[:, 2, :]
                var = fac[:, 3, :]
                tmp = fac[:, 4, :]
                std = fac[:, 5, :]
                nc.vector.tensor_scalar_mul(out=mean, in0=stR[:, 0, :], scalar1=inv_n)
                nc.vector.tensor_mul(out=tmp, in0=mean, in1=mean)
                nc.vector.tensor_scalar_mul(out=var, in0=stR[:, 1, :], scalar1=inv_n)
                nc.vector.tensor_sub(out=var, in0=var, in1=tmp)
                nc.vector.tensor_scalar_add(out=var, in0=var, scalar1=EPS)
                nc.scalar.activation(out=std, in_=var, func=AF.Sqrt)
                nc.vector.reciprocal(out=tmp, in_=std)
                nc.vector.tensor_mul(
                    out=fac[:, 0, :], in0=tmp, in1=bnp[:, sc_col * CC : (sc_col + 1) * CC]
                )
                nc.vector.tensor_mul(out=tmp, in0=mean, in1=fac[:, 0, :])
                nc.vector.tensor_sub(
                    out=fac[:, 1, :], in0=bnp[:, bi_col * CC : (bi_col + 1) * CC], in1=tmp
                )

            bn_factors(st1r, fac1, 0, 1)

            # ---- normalize y in place (y -> yn) ----
            for dc in range(CC):
                for isc in range(ns):
                    ysl = y_sb[dc][:, isc * SC : (isc + 1) * SC]
                    nc.vector.tensor_scalar(
                        out=ysl,
                        in0=ysl,
                        scalar1=fac1[:, 0, dc : dc + 1],
                        scalar2=fac1[:, 1, dc : dc + 1],
                        op0=ALU.mult,
                        op1=ALU.add,
                    )

            # ---- PASS B: z = yn @ ck ----
            psB_ctx = tc.tile_pool(name="psB", bufs=4, space="PSUM")
            pb_ctx = tc.tile_pool(name="pb", bufs=3)
            psB = psB_ctx.__enter__()
            pb = pb_ctx.__enter__()
            for isc in range(ns):
                s0 = isc * SC
                for oc in range(CC):
                    zp = psB.tile([128, SC], F32, name="zp", tag="zp")
                    for cc in range(CC):
                        nc.tensor.matmul(
                            out=zp,
                            lhsT=ck_sb[cc][:, oc * 128 : (oc + 1) * 128],
                            rhs=y_sb[cc][:, s0 : s0 + SC],
                            start=(cc == 0),
                            stop=(cc == CC - 1),
                        )
                    zsl = z_sb[oc][:, s0 : s0 + SC]
                    nc.scalar.activation(
                        out=zsl,
                        in_=zp,
                        func=AF.Identity,
                        bias=zero_bias[:, 0:1],
                        accum_out=zsum[:, oc, isc : isc + 1],
                    )
                    tb2 = pb.tile([128, SC], BF16, name="tb2", tag="tb2")
                    nc.vector.scalar_tensor_tensor(
                        out=tb2,
                        in0=zsl,
                        scalar=1.0,
                        in1=zsl,
                        op0=ALU.mult,
                        op1=ALU.mult,
                        accum_out=zsq[:, oc, isc : isc + 1],
                    )

            psB_ctx.__exit__(None, None, None)
            pb_ctx.__exit__(None, None, None)

            # ---- BN2 stats all-reduce ----
            for oc in range(CC):
                nc.vector.reduce_sum(out=st2[:, 0, oc : oc + 1], in_=zsum[:, oc, :], axis=mybir.AxisListType.X)
                nc.vector.reduce_sum(out=st2[:, 1, oc : oc + 1], in_=zsq[:, oc, :], axis=mybir.AxisListType.X)
            nc.sync.dma_start(out=bounce2i[:, :], in_=st2[:, :, :])
            nc.gpsimd.collective_compute(
                "AllReduce",
                ALU.add,
                replica_groups=[list(range(n_cores))],
                ins=[bounce2i.opt()],
                outs=[bounce2o.opt()],
            )
            nc.sync.dma_start(out=st2r[:, :, :], in_=bounce2o[:, :])
            bn_factors(st2r, fac2, 2, 3)

            # ---- FINAL: out = yn + gelu(z*rg2 + bmr2) ----
            pf = ctx.enter_context(tc.tile_pool(name="pf", bufs=3))
            for isc in range(ns):
                s0 = isc * SC
                for oc in range(CC):
                    g = pf.tile([128, SC], BF16, name="g", tag="g")
                    nc.scalar.activation(
                        out=g,
                        in_=z_sb[oc][:, s0 : s0 + SC],
                        func=gelu_fn,
                        scale=fac2[:, 0, oc : oc + 1],
                        bias=fac2[:, 1, oc : oc + 1],
                    )
                    o32 = pf.tile([128, SC], F32, name="o32", tag="o32")
                    nc.vector.tensor_add(out=o32, in0=y_sb[oc][:, s0 : s0 + SC], in1=g)
                    nc.sync.dma_start(
                        out=out_ext[oc * 128 : (oc + 1) * 128, s0 : s0 + SC], in_=o32
                    )

    nc.compile()
    return nc


def _host_prep(x, weights, bn1_scale, bn1_bias, conv_kernel, bn2_scale, bn2_bias, s_len=S, n_cores=N_CORES):
    """Pre-layout everything on the host; returns per-core in_maps."""
    bf = ml_dtypes.bfloat16
    # x: (B, S, C) -> per-core (C, S) bf16
    xts = [np.ascontiguousarray(x[i].T).astype(bf) for i in range(n_cores)]
    # weights: (D, C, K) -> [cc, c(128), k, d] bf16 (matmul lhsT layout)
    wt = np.ascontiguousarray(np.transpose(weights, (1, 2, 0))).astype(bf)  # (C, K, D)
    wt = wt.reshape(CC, 128, K, C)
    # conv_kernel: (C, O) -> [cc, c(128), o]
    ck = np.ascontiguousarray(conv_kernel).astype(bf).reshape(CC, 128, C)
    # bn params -> [128, 4*CC] f32: cols [g1 | b1 | g2 | b2], each [128, CC]
    def pack(p):
        return np.ascontiguousarray(p.reshape(CC, 128).T)
    bnp = np.concatenate(
        [pack(bn1_scale), pack(bn1_bias), pack(bn2_scale), pack(bn2_bias)], axis=1
    ).astype(np.float32)
    in_maps = [
        {"xt": xts[i], "wt": wt, "ck": ck, "bnp": bnp} for i in range(n_cores)
    ]
    return in_maps


_NC_CACHE = {}


def kernel(x, weights, bn1_scale, bn1_bias, conv_kernel, bn2_scale, bn2_bias):
    x = np.asarray(x, dtype=np.float32)
    weights = np.asarray(weights, dtype=np.float32)
    bn1_scale = np.asarray(bn1_scale, dtype=np.float32)
    bn1_bias = np.asarray(bn1_bias, dtype=np.float32)
    conv_kernel = np.asarray(conv_kernel, dtype=np.float32)
    bn2_scale = np.asarray(bn2_scale, dtype=np.float32)
    bn2_bias = np.asarray(bn2_bias, dtype=np.float32)

    if "nc" not in _NC_CACHE:
        _NC_CACHE["nc"] = build()
    nc = _NC_CACHE["nc"]

    in_maps = _host_prep(x, weights, bn1_scale, bn1_bias, conv_kernel, bn2_scale, bn2_bias)
    res = run_bass_kernel_spmd(nc, in_maps, list(range(N_CORES)))
    out = np.stack([res.results[i]["out"].T for i in range(N_CORES)], axis=0)
    return np.ascontiguousarray(out.astype(np.float32))


# revision 7
# speedup vs baseline: 1.0707x; 1.0225x over previous
"""Distributed Trainium2 kernel for nn_Convblock_72919954751797.

Reference computation (per full input):
    x: (B=8, S=4096, C=512) f32
    w = tanh(einsum('bsc,dck->bkds', x, weights))        # content-dependent taps
    y = x + sum_k shift(x, k-3) * w[k]                   # dynamic depthwise conv
    y = BN1(y)  (stats over (B,S))
    z = gelu_tanh(BN2(y @ conv_kernel))
    out = y + z

Sharding: pure data-parallel over batch (1 sample per core); the only
cross-core traffic is two 4KB AllReduces for the BatchNorm statistics.

On-chip layout is (channel, seq) with channel on partitions. The host
pre-transposes x to (C, S) bf16 and pre-arranges weights into matmul lhsT
layout so the cores do zero layout work. x is kept in two SBUF copies
offset by one column so every shifted dynamic-conv read is 4B-aligned
(DVE 2x packed mode). BN1 is folded into the 1x1 conv weights
(W' = diag(r*gamma) W, bias folded into the BN2+gelu activation bias) so
PASS B starts immediately after the first all-reduce.
"""

import sys
import types

sys.path.insert(0, "/opt/trn_rl_repo")

import numpy as np
import ml_dtypes

import concourse.bass as bass
import concourse.tile as tile
from concourse import bacc, mybir
from concourse.bass_utils import run_bass_kernel_spmd

AF = mybir.ActivationFunctionType
ALU = mybir.AluOpType
BF16 = mybir.dt.bfloat16
F32 = mybir.dt.float32

N_CORES = 8
B, S, C, K = 8, 4096, 512, 7
EPS = 1e-5
CC = C // 128          # channel chunks of 128 partitions
SC = 512               # seq-chunk (matmul moving dim)
PAD = 4                # left pad for shift halo (>=3)
HALF = K // 2
SH = 2                 # seq-chunks per PASS-A round group
FB = 4                 # seq-chunks per FINAL block


def build(s_len=S, n_cores=N_CORES, gelu_fn=None):
    if gelu_fn is None:
        gelu_fn = AF.Gelu_apprx_tanh
    ns = s_len // SC
    inv_n = 1.0 / (n_cores * s_len)

    nc = bacc.Bacc(None, target_bir_lowering=False, num_devices=n_cores)

    xt_ext = nc.declare_dram_parameter("xt", [C, s_len], BF16, isOutput=False)
    wt_ext = nc.declare_dram_parameter("wt", [CC, 128, K, C], BF16, isOutput=False)
    ck_ext = nc.declare_dram_parameter("ck", [CC, 128, C], BF16, isOutput=False)
    bnp_ext = nc.declare_dram_parameter("bnp", [128, 4 * CC], F32, isOutput=False)
    out_ext = nc.declare_dram_parameter("out", [C, s_len], F32, isOutput=True)

    xw = PAD + s_len + PAD

    with tile.TileContext(nc) as tc:
        import contextlib

        ctx = contextlib.ExitStack()
        with ctx:
            pers = ctx.enter_context(tc.tile_pool(name="pers", bufs=1))
            dram = ctx.enter_context(tc.tile_pool(name="dram", bufs=1, space="DRAM"))

            # ---- persistent SBUF tensors ----
            x_cs = [pers.tile([128, xw], BF16, name=f"x_cs{i}", tag=f"x{i}") for i in range(CC)]
            # x_odd[:, j] == x_cs[:, j+1]: aligned reads for even-k shifts
            x_od = [pers.tile([128, xw], BF16, name=f"x_od{i}", tag=f"xo{i}") for i in range(CC)]
            w_sb = [pers.tile([128, K, C], BF16, name=f"w_sb{i}", tag=f"w{i}") for i in range(CC)]
            ck_sb = [pers.tile([128, C], BF16, name=f"ck_sb{i}", tag=f"ck{i}") for i in range(CC)]
            ckf = [pers.tile([128, C], BF16, name=f"ckf{i}", tag=f"ckf{i}") for i in range(CC)]
            y_sb = [pers.tile([128, s_len], BF16, name=f"y_sb{i}", tag=f"y{i}") for i in range(CC)]
            z_sb = [pers.tile([128, s_len], BF16, name=f"z_sb{i}", tag=f"z{i}") for i in range(CC)]
            bnp = pers.tile([128, 4 * CC], F32, name="bnp", tag="bnp")
            ysum = pers.tile([128, CC, ns], F32, name="ysum", tag="ysum")
            ysq = pers.tile([128, CC, ns], F32, name="ysq", tag="ysq")
            zsum = pers.tile([128, CC, ns], F32, name="zsum", tag="zsum")
            zsq = pers.tile([128, CC, ns], F32, name="zsq", tag="zsq")
            st1 = pers.tile([128, 2, CC], F32, name="st1", tag="st1")
            st1r = pers.tile([128, 2, CC], F32, name="st1r", tag="st1r")
            st2 = pers.tile([128, 2, CC], F32, name="st2", tag="st2")
            st2r = pers.tile([128, 2, CC], F32, name="st2r", tag="st2r")
            fac1 = pers.tile([128, 6, CC], F32, name="fac1", tag="fac1")
            fac2 = pers.tile([128, 6, CC], F32, name="fac2", tag="fac2")
            bmb = pers.tile([128, CC], BF16, name="bmb", tag="bmb")
            bconv = pers.tile([128, CC], F32, name="bconv", tag="bconv")
            badj = pers.tile([128, CC], F32, name="badj", tag="badj")
            zero_bias = pers.tile([128, 1], F32, name="zero_bias", tag="zb")

            bounce1i = dram.tile([128, 2 * CC], F32, name="bounce1i", tag="b1i")
            bounce1o = dram.tile([128, 2 * CC], F32, name="bounce1o", tag="b1o")
            bounce2i = dram.tile([128, 2 * CC], F32, name="bounce2i", tag="b2i")
            bounce2o = dram.tile([128, 2 * CC], F32, name="bounce2o", tag="b2o")

            # ---- loads ----
            nc.sync.dma_start(out=bnp, in_=bnp_ext[:, :])
            for cc in range(CC):
                nc.vector.memset(x_cs[cc][:, 0:PAD], 0)
                nc.vector.memset(x_cs[cc][:, PAD + s_len : xw], 0)
                nc.vector.memset(x_od[cc][:, 0 : PAD - 1], 0)
                nc.vector.memset(x_od[cc][:, PAD - 1 + s_len : xw], 0)
                nc.sync.dma_start(
                    out=x_cs[cc][:, PAD : PAD + s_len],
                    in_=xt_ext[cc * 128 : (cc + 1) * 128, :],
                )
                nc.sync.dma_start(
                    out=x_od[cc][:, PAD - 1 : PAD - 1 + s_len],
                    in_=xt_ext[cc * 128 : (cc + 1) * 128, :],
                )
                nc.sync.dma_start(out=w_sb[cc], in_=wt_ext[cc])
                nc.sync.dma_start(out=ck_sb[cc], in_=ck_ext[cc])
            nc.vector.memset(zero_bias, 0.0)

            # aligned slice helper: for shift d = k - HALF, absolute start
            # in x_cs is PAD + s0 + d; if that is odd, read x_od at start-1.
            def xsl(cc, s0, k, width=SC):
                st = PAD + s0 + k - HALF
                if st % 2 == 0:
                    return x_cs[cc][:, st : st + width]
                return x_od[cc][:, st - 1 : st - 1 + width]

            # ---- PASS A: w_pre matmul + tanh + dynamic conv -> y, stats ----
            pa_ctx = tc.tile_pool(name="pa", bufs=2)
            psA_ctx = tc.tile_pool(name="psA", bufs=3, space="PSUM")
            pa = pa_ctx.__enter__()
            psA = psA_ctx.__enter__()

            nsh = (ns + SH - 1) // SH
            sh_chunks = [list(range(h * SH, min((h + 1) * SH, ns))) for h in range(nsh)]
            for dc in range(CC):
                for chunks in sh_chunks:
                    nch = len(chunks)
                    wt_t = pa.tile([128, K, SH, SC], BF16, name="wt_t", tag="wt_t")
                    for k in range(K):
                        wp = psA.tile([128, SH, SC], F32, name="wp", tag="wp")
                        for cc in range(CC):
                            for j, isc in enumerate(chunks):
                                s0 = isc * SC
                                nc.tensor.matmul(
                                    out=wp[:, j, :],
                                    lhsT=w_sb[cc][:, k, dc * 128 : (dc + 1) * 128],
                                    rhs=x_cs[cc][:, PAD + s0 : PAD + s0 + SC],
                                    start=(cc == 0),
                                    stop=(cc == CC - 1),
                                )
                        nc.scalar.activation(
                            out=wt_t[:, k, 0:nch, :],
                            in_=wp[:, 0:nch, :],
                            func=AF.Tanh,
                        )
                    for j, isc in enumerate(chunks):
                        s0 = isc * SC
                        ta = pa.tile([128, SC], BF16, name="ta", tag="ta")
                        tb = pa.tile([128, SC], BF16, name="tb", tag="tb")
                        nc.vector.tensor_mul(out=ta, in0=xsl(dc, s0, 0), in1=wt_t[:, 0, j, :])
                        for k in range(1, K):
                            nc.vector.tensor_mul(out=tb, in0=xsl(dc, s0, k), in1=wt_t[:, k, j, :])
                            nc.vector.tensor_add(out=ta, in0=ta, in1=tb)
                        ysl = y_sb[dc][:, s0 : s0 + SC]
                        nc.vector.scalar_tensor_tensor(
                            out=ysl,
                            in0=ta,
                            scalar=1.0,
                            in1=x_cs[dc][:, PAD + s0 : PAD + s0 + SC],
                            op0=ALU.mult,
                            op1=ALU.add,
                            accum_out=ysum[:, dc, isc : isc + 1],
                        )
                        nc.vector.scalar_tensor_tensor(
                            out=tb,
                            in0=ysl,
                            scalar=1.0,
                            in1=ysl,
                            op0=ALU.mult,
                            op1=ALU.mult,
                            accum_out=ysq[:, dc, isc : isc + 1],
                        )

            psA_ctx.__exit__(None, None, None)
            pa_ctx.__exit__(None, None, None)

            # ---- BN1 stats all-reduce ----
            for dc in range(CC):
                nc.vector.reduce_sum(out=st1[:, 0, dc : dc + 1], in_=ysum[:, dc, :], axis=mybir.AxisListType.X)
                nc.vector.reduce_sum(out=st1[:, 1, dc : dc + 1], in_=ysq[:, dc, :], axis=mybir.AxisListType.X)
            nc.sync.dma_start(out=bounce1i[:, :], in_=st1[:, :, :])
            nc.gpsimd.collective_compute(
                "AllReduce",
                ALU.add,
                replica_groups=[list(range(n_cores))],
                ins=[bounce1i.opt()],
                outs=[bounce1o.opt()],
            )
            nc.sync.dma_start(out=st1r[:, :, :], in_=bounce1o[:, :])

            # factors: mean = sum/n ; var = sq/n - mean^2 ; rg = scale/sqrt(var+eps)
            # bmr = bias - mean*rg    (fac[:,0,:]=rg, fac[:,1,:]=bmr)
            def bn_factors(stR, fac, sc_col, bi_col):
                mean = fac[:, 2, :]
                var = fac[:, 3, :]
                tmp = fac[:, 4, :]
                std = fac[:, 5, :]
                nc.vector.tensor_scalar_mul(out=mean, in0=stR[:, 0, :], scalar1=inv_n)
                nc.vector.tensor_mul(out=tmp, in0=mean, in1=mean)
                nc.vector.tensor_scalar_mul(out=var, in0=stR[:, 1, :], scalar1=inv_n)
                nc.vector.tensor_sub(out=var, in0=var, in1=tmp)
                nc.vector.tensor_scalar_add(out=var, in0=var, scalar1=EPS)
                nc.scalar.activation(out=std, in_=var, func=AF.Sqrt)
                nc.vector.reciprocal(out=tmp, in_=std)
                nc.vector.tensor_mul(
                    out=fac[:, 0, :], in0=tmp, in1=bnp[:, sc_col * CC : (sc_col + 1) * CC]
                )
                nc.vector.tensor_mul(out=tmp, in0=mean, in1=fac[:, 0, :])
                nc.vector.tensor_sub(
                    out=fac[:, 1, :], in0=bnp[:, bi_col * CC : (bi_col + 1) * CC], in1=tmp
                )

            bn_factors(st1r, fac1, 0, 1)

            # fold BN1 into conv: W' = diag(rg1) @ W ; bconv_o = sum_c bmr1_c W[c,o]
            for cc in range(CC):
                nc.vector.tensor_scalar_mul(
                    out=ckf[cc], in0=ck_sb[cc], scalar1=fac1[:, 0, cc : cc + 1]
                )
            nc.vector.tensor_copy(out=bmb, in_=fac1[:, 1, :])

            # ---- PASS B: z = y @ W' + bconv (z == BN1(y) @ W) ----
            psB_ctx = tc.tile_pool(name="psB", bufs=3, space="PSUM")
            pb_ctx = tc.tile_pool(name="pb", bufs=3)
            psB = psB_ctx.__enter__()
            pb = pb_ctx.__enter__()

            for oc in range(CC):
                bp = psB.tile([128, 1], F32, name="bp", tag="bp", bufs=1)
                for cc in range(CC):
                    nc.tensor.matmul(
                        out=bp,
                        lhsT=ck_sb[cc][:, oc * 128 : (oc + 1) * 128],
                        rhs=bmb[:, cc : cc + 1],
                        start=(cc == 0),
                        stop=(cc == CC - 1),
                    )
                nc.vector.tensor_copy(out=bconv[:, oc : oc + 1], in_=bp)

            npair = (ns + 1) // 2
            for ip in range(npair):
                chunks = [c for c in (2 * ip, 2 * ip + 1) if c < ns]
                nch = len(chunks)
                s0 = chunks[0] * SC
                for oc in range(CC):
                    zp = psB.tile([128, 2, SC], F32, name="zp", tag="zp")
                    for cc in range(CC):
                        for j, isc in enumerate(chunks):
                            nc.tensor.matmul(
                                out=zp[:, j, :],
                                lhsT=ckf[cc][:, oc * 128 : (oc + 1) * 128],
                                rhs=y_sb[cc][:, isc * SC : (isc + 1) * SC],
                                start=(cc == 0),
                                stop=(cc == CC - 1),
                            )
                    zsl = z_sb[oc][:, s0 : s0 + nch * SC]
                    nc.scalar.activation(
                        out=zsl,
                        in_=zp[:, 0:nch, :],
                        func=AF.Identity,
                        bias=bconv[:, oc : oc + 1],
                        accum_out=zsum[:, oc, ip : ip + 1],
                    )
                    tb2 = pb.tile([128, 2 * SC], BF16, name="tb2", tag="tb2")
                    nc.vector.scalar_tensor_tensor(
                        out=tb2[:, 0 : nch * SC],
                        in0=zsl,
                        scalar=1.0,
                        in1=zsl,
                        op0=ALU.mult,
                        op1=ALU.mult,
                        accum_out=zsq[:, oc, ip : ip + 1],
                    )

            # normalize y in place (y -> yn) for the final residual; runs on
            # DVE during PASS B (waits for the conv reads of each slice).
            for dc in range(CC):
                for isc in range(ns):
                    ysl = y_sb[dc][:, isc * SC : (isc + 1) * SC]
                    nc.vector.tensor_scalar(
                        out=ysl,
                        in0=ysl,
                        scalar1=fac1[:, 0, dc : dc + 1],
                        scalar2=fac1[:, 1, dc : dc + 1],
                        op0=ALU.mult,
                        op1=ALU.add,
                    )

            psB_ctx.__exit__(None, None, None)
            pb_ctx.__exit__(None, None, None)

            # ---- BN2 stats all-reduce ----
            for oc in range(CC):
                nc.vector.reduce_sum(out=st2[:, 0, oc : oc + 1], in_=zsum[:, oc, 0:npair], axis=mybir.AxisListType.X)
                nc.vector.reduce_sum(out=st2[:, 1, oc : oc + 1], in_=zsq[:, oc, 0:npair], axis=mybir.AxisListType.X)
            nc.sync.dma_start(out=bounce2i[:, :], in_=st2[:, :, :])
            nc.gpsimd.collective_compute(
                "AllReduce",
                ALU.add,
                replica_groups=[list(range(n_cores))],
                ins=[bounce2i.opt()],
                outs=[bounce2o.opt()],
            )
            nc.sync.dma_start(out=st2r[:, :, :], in_=bounce2o[:, :])
            bn_factors(st2r, fac2, 2, 3)
            # z stored in z_sb excludes bconv? No: z_sb includes +bconv, and
            # stats were computed on stored z, so factors are consistent.
            # badj = bmr2 (nothing extra: bconv already inside z and stats).

            # ---- FINAL: out = yn + gelu(z*rg2 + bmr2), in FB-chunk blocks ----
            pf_ctx = tc.tile_pool(name="pf", bufs=3)
            pf = pf_ctx.__enter__()
            nblk = (ns + FB - 1) // FB
            for ib in range(nblk):
                c0 = ib * FB
                w = min(FB, ns - c0) * SC
                s0 = c0 * SC
                for oc in range(CC):
                    g = pf.tile([128, FB * SC], BF16, name="g", tag="g")
                    nc.scalar.activation(
                        out=g[:, 0:w],
                        in_=z_sb[oc][:, s0 : s0 + w],
                        func=gelu_fn,
                        scale=fac2[:, 0, oc : oc + 1],
                        bias=fac2[:, 1, oc : oc + 1],
                    )
                    o32 = pf.tile([128, FB * SC], F32, name="o32", tag="o32")
                    nc.vector.tensor_add(
                        out=o32[:, 0:w], in0=y_sb[oc][:, s0 : s0 + w], in1=g[:, 0:w]
                    )
                    nc.sync.dma_start(
                        out=out_ext[oc * 128 : (oc + 1) * 128, s0 : s0 + w],
                        in_=o32[:, 0:w],
                    )
            pf_ctx.__exit__(None, None, None)

    nc.compile()
    return nc


def _host_prep(x, weights, bn1_scale, bn1_bias, conv_kernel, bn2_scale, bn2_bias, s_len=S, n_cores=N_CORES):
    """Pre-layout everything on the host; returns per-core in_maps."""
    bf = ml_dtypes.bfloat16
    xts = [np.ascontiguousarray(x[i].T).astype(bf) for i in range(n_cores)]
    wt = np.ascontiguousarray(np.transpose(weights, (1, 2, 0))).astype(bf)  # (C, K, D)
    wt = wt.reshape(CC, 128, K, C)
    ck = np.ascontiguousarray(conv_kernel).astype(bf).reshape(CC, 128, C)

    def pack(p):
        return np.ascontiguousarray(p.reshape(CC, 128).T)

    bnp = np.concatenate(
        [pack(bn1_scale), pack(bn1_bias), pack(bn2_scale), pack(bn2_bias)], axis=1
    ).astype(np.float32)
    in_maps = [
        {"xt": xts[i], "wt": wt, "ck": ck, "bnp": bnp} for i in range(n_cores)
    ]
    return in_maps


_NC_CACHE = {}


def kernel(x, weights, bn1_scale, bn1_bias, conv_kernel, bn2_scale, bn2_bias):
    x = np.asarray(x, dtype=np.float32)
    weights = np.asarray(weights, dtype=np.float32)
    bn1_scale = np.asarray(bn1_scale, dtype=np.float32)
    bn1_bias = np.asarray(bn1_bias, dtype=np.float32)
    conv_kernel = np.asarray(conv_kernel, dtype=np.float32)
    bn2_scale = np.asarray(bn2_scale, dtype=np.float32)
    bn2_bias = np.asarray(bn2_bias, dtype=np.float32)

    if "nc" not in _NC_CACHE:
        _NC_CACHE["nc"] = build()
    nc = _NC_CACHE["nc"]

    in_maps = _host_prep(x, weights, bn1_scale, bn1_bias, conv_kernel, bn2_scale, bn2_bias)
    res = run_bass_kernel_spmd(nc, in_maps, list(range(N_CORES)))
    out = np.stack([res.results[i]["out"].T for i in range(N_CORES)], axis=0)
    return np.ascontiguousarray(out.astype(np.float32))
